# revision 36
# baseline (speedup 1.0000x reference)
"""MoE routing kernel for Trainium2, 8 NeuronCores.

Strategy (two device launches, host does routing bookkeeping):

  Launch 1 (data-parallel gating): each core computes the gating
  logits for its 1/8 token shard as logits^T = gate_w @ x^T with the
  gate weights stationary (16 columns) and tokens moving (N=512),
  in bf16 with f32 PSUM accumulation. Output: raw logits [E, B/8].
  Host: softmax / top-k / capacity planning from the device logits;
  rows whose top-k decision margin is below DELTA are recomputed in
  f32 so routing matches an f32 reference. Tokens are gathered per
  expert and pre-scaled by their gate values (linearity: g*(xW) =
  (g x)W), then packed bf16 in the [P, KT, C] SBUF layout.

  Launch 2 (expert-parallel): experts are paired big-with-small onto
  cores (2 per core) with per-slot capacities C0 >= C1. Per expert,
  the weight tile W[k, n] is stationary (bf16 -> fast weight load)
  and token columns are moving, in n-outer waves of <= 3 PSUM banks
  (k-outer inside a wave so the PE chases the input DMA stream;
  the first expert's first two waves run as one 6-bank group to
  keep the PE fed during the initial stream). Each output row-tile
  y^T[n] [128, C] is written bf16 as soon as its wave finishes, so
  output DMA overlaps compute instead of forming a tail; the host
  scatter-adds the compact outputs into the final [B, DOUT] f32.

All expert FLOPs and the gating matmul run on device; the host does
index bookkeeping, gate-value scaling, and the final unshard.
"""
import numpy as np
from contextlib import ExitStack

from ml_dtypes import bfloat16 as np_bf16

import concourse.mybir as mybir
from concourse import bacc, tile
from concourse.bass_utils import run_bass_kernel_spmd

NCORES = 8
P = 128
NF = 512
F32 = mybir.dt.float32
BF16 = mybir.dt.bfloat16
ACT_COPY = mybir.ActivationFunctionType.Copy

# margin (in logit units) below which the host re-computes a row's
# gating exactly in f32; device logits carry ~2e-3 abs bf16 error
DELTA = 0.02

# test-harness knobs
TRACE = False
LAST_EXEC_NS = []
LAST_RESULTS = {}

_cache = {}


def _warmup_pe(nc, pool, ps_pool, n_mm, tag="ps"):
    """Dummy bf16 matmuls on scratch data, issued at kernel start so the
    PE's HAM clock-gate reaches 2.4 GHz while the input DMAs stream in."""
    wt = pool.tile([P, NF], BF16, name="warm_sb")
    nc.gpsimd.memset(wt[:], 1.0)
    wp = ps_pool.tile([P, NF], F32, name="warm_ps", tag=tag)
    for _ in range(n_mm):
        nc.tensor.matmul(wp[:], wt[:, :P], wt[:], start=True, stop=True)
    return wt, wp


def _warmup_act(nc, pool):
    """Load the scalar engine's activation table (Copy) during the input
    DMA shadow so the first real ACTIVATE doesn't pay the ~1.3us table
    load. Emit AFTER the input dma_starts so it doesn't delay their
    descriptor issue on the scalar engine."""
    aw = pool.tile([1, 8], F32, name="warm_act")
    nc.vector.memset(aw[:], 0.0)
    nc.scalar.activation(aw[:], aw[:], ACT_COPY)


def _build_gating(Bloc, DIN, E):
    """logits^T [E, Bloc] = gate_w @ x^T in bf16, f32 accumulation.

    Inputs : xP  [P, KT, Bloc] bf16 (x^T packed: (p,k,b) = x[b, k*P+p])
             gwP [P, KT, E]    bf16 (gate_w^T packed likewise)
    Output : lgT [E, Bloc] f32
    """
    key = ("gate", Bloc, DIN, E)
    if key in _cache:
        return _cache[key]
    KT = DIN // P
    TT = Bloc // NF
    assert Bloc % NF == 0 and E <= P
    nc = bacc.Bacc("TRN2", target_bir_lowering=False, debug=False,
                   num_devices=NCORES)
    xP = nc.dram_tensor("xP", [P, KT, Bloc], BF16, kind="ExternalInput")
    gwP = nc.dram_tensor("gwP", [P, KT, E], BF16, kind="ExternalInput")
    lgT = nc.dram_tensor("lgT", [E, Bloc], F32, kind="ExternalOutput")

    with tile.TileContext(nc) as tc:
        with ExitStack() as ctx:
            const = ctx.enter_context(tc.tile_pool(name="const", bufs=1))
            ps = ctx.enter_context(tc.tile_pool(name="ps", bufs=1,
                                                space="PSUM"))
            _warmup_pe(nc, const, ps, 8)
            gw_t = const.tile([P, KT, E], BF16)
            nc.scalar.dma_start(gw_t[:], gwP[:])
            # x split by (k-half, token-half): token-half t0 lands,
            # computes, and writes back while t1's data still streams,
            # so the input-receipt -> MM -> output-receipt chain is
            # pipelined per token-half instead of paid once at the end
            H = KT // 2
            x_ts = []
            for t in range(TT):
                xl = const.tile([P, H, NF], BF16, name=f"xl{t}")
                nc.sync.dma_start(xl[:], xP[:, :H, t * NF:(t + 1) * NF])
                xh = const.tile([P, H, NF], BF16, name=f"xh{t}")
                nc.scalar.dma_start(xh[:],
                                    xP[:, H:, t * NF:(t + 1) * NF])
                x_ts.append((xl, xh))
            lg_sb = const.tile([E, Bloc], F32)
            for t in range(TT):
                xl, xh = x_ts[t]
                pst = ps.tile([E, NF], F32, name=f"g{t}", tag=f"g{t}")
                for k in range(KT):
                    xt, kk = (xl, k) if k < H else (xh, k - H)
                    nc.tensor.matmul(
                        pst[:], gw_t[:, k], xt[:, kk],
                        start=(k == 0), stop=(k == KT - 1))
                nc.vector.tensor_copy(lg_sb[:, t * NF:(t + 1) * NF],
                                      pst[:])
                nc.sync.dma_start(lgT[:, t * NF:(t + 1) * NF],
                                  lg_sb[:, t * NF:(t + 1) * NF])
    nc.compile()
    _cache[key] = nc
    return nc


def _ksplits(KT):
    """Descriptor schedule for the contraction dim: k0 and k1 alone
    (their DMA-completion receipts fire early, so the PE starts as soon
    as the warmup ends), then pairs."""
    if KT < 4:
        return [(k, 1) for k in range(KT)]
    return [(0, 1), (1, 1)] + [(k, 2) for k in range(2, KT, 2)]


def _xg_ap(entry, kk, c0, cl):
    """Slice a (possibly chunk-split) xg tile entry."""
    if isinstance(entry, tuple):
        xa, xb, ch0 = entry
        if c0 >= ch0:
            return xb[:, kk, c0 - ch0:c0 - ch0 + cl]
        return xa[:, kk, c0:c0 + cl]
    return entry[:, kk, c0:c0 + cl]


def _chunks(C):
    """Split C token columns into near-equal chunks of <= NF (PSUM bank
    limit), multiples of 4 except possibly the last."""
    n = -(-C // NF)
    base = -(-(-(-C // n)) // 4) * 4
    out = []
    c0 = 0
    while c0 < C:
        cl = min(base, C - c0)
        out.append((c0, cl))
        c0 += cl
    return out


def _build_expert(Cs, DIN, DOUT, EPC, has_eb):
    """Per-core expert compute: y_e^T = W_e^T @ xg_e for each of the
    core's EPC experts, where xg_e is the gathered, gate-scaled token
    matrix (capacity Cs[j], transposed). Weights are the stationary
    operand (bf16 -> FWL); tokens move in chunks of <= 512 columns.

    Inputs : xgP{j} [P, KT, Cs[j]] bf16 (packed (p,k,c) = xg[k*P+p, c])
             wP     [EPC, P, KT, DOUT] bf16 (packed likewise)
             bP     [EPC, 1, DOUT] bf16, gvP{j} [1, Cs[j]] bf16 (if bias)
    Output : yT{j}  [DOUT, Cs[j]] bf16
    """
    key = ("exp", tuple(Cs), DIN, DOUT, EPC, has_eb)
    if key in _cache:
        return _cache[key]
    KT = DIN // P
    NT = DOUT // P
    nc = bacc.Bacc("TRN2", target_bir_lowering=False, debug=False,
                   num_devices=NCORES)
    xg_d = [nc.dram_tensor(f"xgP{j}", [P, KT, Cs[j]], BF16,
                           kind="ExternalInput") for j in range(EPC)]
    w_d = nc.dram_tensor("wP", [EPC, P, KT, DOUT], BF16,
                         kind="ExternalInput")
    y_d = [nc.dram_tensor(f"yT{j}", [DOUT, Cs[j]], BF16,
                          kind="ExternalOutput") for j in range(EPC)]
    if has_eb:
        b_d = nc.dram_tensor("bP", [EPC, 1, DOUT], BF16,
                             kind="ExternalInput")
        gv_d = [nc.dram_tensor(f"gvP{j}", [1, Cs[j]], BF16,
                               kind="ExternalInput") for j in range(EPC)]

    with tile.TileContext(nc) as tc:
        with ExitStack() as ctx:
            xgp = ctx.enter_context(tc.tile_pool(name="xg", bufs=2))
            wp = ctx.enter_context(tc.tile_pool(name="w", bufs=2))
            outp = ctx.enter_context(tc.tile_pool(name="out", bufs=2))
            psp = ctx.enter_context(tc.tile_pool(name="ps", bufs=8,
                                                 space="PSUM"))
            warm = ctx.enter_context(tc.tile_pool(name="warm", bufs=1))
            if has_eb:
                bp = ctx.enter_context(tc.tile_pool(name="b", bufs=2))
            _warmup_pe(nc, warm, psp, 8)

            # pre-issue all input DMAs (inputs for expert j+1 stream
            # while expert j computes; outputs are emitted later so the
            # FIFO HWDGE rings never block a later expert's inputs).
            # Per-k descriptors in consumption order so the PE can chase
            # the stream; xg on the sync ring, w split scalar/gpsimd so
            # three queues pull HBM in parallel.
            splits = _ksplits(KT)
            k2s = {}
            for si, (k0, kl) in enumerate(splits):
                for kk in range(kl):
                    k2s[k0 + kk] = (si, kk)
            tiles = []
            for j in range(EPC):
                # per-split tiles: exact deps let the PE chase the DMA
                # stream split by split instead of waiting for the whole
                # tensor; xg on the sync ring, w on the scalar ring,
                # both in consumption order, smallest splits first
                xg_q = []
                w_q = []
                ch0 = _chunks(Cs[j])[0][1]
                for si, (k0, kl) in enumerate(splits):
                    if j == 0 and si == 0:
                        # the very first MM only needs k0 of the first
                        # token chunk: land it (and its completion
                        # receipt) as early as possible
                        xa = xgp.tile([P, kl, ch0], BF16, tag="xg0a",
                                      name=f"xg{j}_0a")
                        nc.sync.dma_start(xa[:],
                                          xg_d[j][:, k0:k0 + kl, :ch0])
                        xb = xgp.tile([P, kl, Cs[j] - ch0], BF16,
                                      tag="xg0b", name=f"xg{j}_0b")
                        nc.sync.dma_start(xb[:],
                                          xg_d[j][:, k0:k0 + kl, ch0:])
                        xg_q.append((xa, xb, ch0))
                    else:
                        xq = xgp.tile([P, kl, Cs[j]], BF16,
                                      tag=f"xg{si}", name=f"xg{j}_{si}")
                        nc.sync.dma_start(xq[:], xg_d[j][:, k0:k0 + kl])
                        xg_q.append(xq)
                    wq = wp.tile([P, kl, DOUT], BF16, tag=f"w{si}",
                                 name=f"w{j}_{si}")
                    nc.scalar.dma_start(wq[:], w_d[j, :, k0:k0 + kl])
                    w_q.append(wq)
                if has_eb:
                    b_t = bp.tile([1, DOUT], BF16, tag="b", name=f"b{j}")
                    nc.gpsimd.dma_start(b_t[:], b_d[j])
                    gv_t = bp.tile([1, Cs[j]], BF16, tag="gv",
                                   name=f"gv{j}")
                    nc.gpsimd.dma_start(gv_t[:], gv_d[j][:])
                    tiles.append((xg_q, w_q, b_t, gv_t))
                else:
                    tiles.append((xg_q, w_q, None, None))
            _warmup_act(nc, warm)

            for j in range(EPC):
                xg_q, w_q, b_t, gv_t = tiles[j]
                C = Cs[j]
                chs = _chunks(C)
                out_t = outp.tile([P, NT, C], BF16, tag="out",
                                  name=f"out{j}")
                ev = 0
                # n-outer waves: each output row-tile finishes early and
                # streams out while later waves compute (no bulk tail).
                # The first expert's first two waves run as one 6-bank
                # group so the PE has 2x the work per arriving k-split
                # while it chases the initial input DMA stream.
                if j == 0 and len(chs) <= 3:
                    groups = [[0, 1]] + [[n] for n in range(2, NT)]
                else:
                    groups = [[n] for n in range(NT)]
                for grp in groups:
                    pss = {n: [psp.tile([P, NF], F32, tag="ps",
                                        name=f"ps{j}_{n}_{ci}")
                               for ci in range(len(chs))]
                           for n in grp}
                    for k in range(KT):
                        si, kk = k2s[k]
                        for n in grp:
                            for ci, (c0, cl) in enumerate(chs):
                                nc.tensor.matmul(
                                    pss[n][ci][:, :cl],
                                    w_q[si][:, kk, n * P:(n + 1) * P],
                                    _xg_ap(xg_q[si], kk, c0, cl),
                                    start=(k == 0),
                                    stop=(k == KT - 1 and not has_eb))
                    # the very last wave's output is on the kernel's
                    # critical tail: fire it per-chunk so each block
                    # leaves as soon as its eviction lands
                    last_wave = (j == EPC - 1 and grp[-1] == NT - 1)
                    for n in grp:
                        for ci, (c0, cl) in enumerate(chs):
                            if has_eb:
                                nc.tensor.matmul(
                                    pss[n][ci][:, :cl],
                                    b_t[:1, n * P:(n + 1) * P],
                                    gv_t[:1, c0:c0 + cl],
                                    start=False, stop=True)
                            dst = out_t[:, n, c0:c0 + cl]
                            if ev % 2 == 0:
                                nc.vector.tensor_copy(dst,
                                                      pss[n][ci][:, :cl])
                            else:
                                nc.scalar.activation(
                                    dst, pss[n][ci][:, :cl], ACT_COPY)
                            ev += 1
                            if last_wave and n == NT - 1:
                                deng = (nc.sync, nc.scalar)[ci % 2]
                                deng.dma_start(
                                    y_d[j][n * P:(n + 1) * P,
                                           c0:c0 + cl], dst)
                        if not (last_wave and n == NT - 1):
                            deng = (nc.sync, nc.scalar, nc.gpsimd)[n % 3]
                            deng.dma_start(y_d[j][n * P:(n + 1) * P, :],
                                           out_t[:, n, :])
    nc.compile()
    _cache[key] = nc
    return nc


def _run(nc, in_maps):
    kw = {}
    if TRACE:
        kw["trace"] = True
    res = run_bass_kernel_spmd(nc, in_maps, list(range(NCORES)), **kw)
    if TRACE:
        LAST_EXEC_NS.append(res.exec_time_ns)
        LAST_RESULTS["last"] = res
    return res.results


def _pack(mat, KT):
    """[D, N] -> [P, KT, N] with (p, k, n) = mat[k*P+p, n]."""
    D, N = mat.shape
    return np.ascontiguousarray(
        mat.reshape(KT, P, N).transpose(1, 0, 2))


def kernel(x, gate_w, gate_b, expert_w, expert_b, topk):
    x = np.ascontiguousarray(np.asarray(x, dtype=np.float32))
    gate_w = np.asarray(gate_w, dtype=np.float32)
    gate_b = np.asarray(gate_b, dtype=np.float32)
    expert_w = np.asarray(expert_w, dtype=np.float32)
    expert_b = np.asarray(expert_b, dtype=np.float32)
    topk = int(topk)

    B, DIN = x.shape
    E, _, DOUT = expert_w.shape
    assert B % (NCORES * NF) == 0 and DIN % P == 0 and DOUT % (4 * P) == 0
    Bloc = B // NCORES
    EPC = E // NCORES
    assert EPC * NCORES == E and E <= P
    KT = DIN // P
    has_eb = bool(np.any(expert_b))

    # ---- launch 1: gating logits (data-parallel over tokens) ----
    nc1 = _build_gating(Bloc, DIN, E)
    gwP = _pack(gate_w.T.astype(np_bf16), KT)
    xb = x.astype(np_bf16)
    in1 = []
    for c in range(NCORES):
        xP = _pack(np.ascontiguousarray(
            xb[c * Bloc:(c + 1) * Bloc].T), KT)
        in1.append({"xP": xP, "gwP": gwP})
    r1 = _run(nc1, in1)
    logits = np.concatenate(
        [r1[c]["lgT"] for c in range(NCORES)], axis=1).T
    logits = np.ascontiguousarray(logits) + gate_b[None, :]

    # ---- host: exact re-gating for ambiguous rows ----
    if topk < E:
        part = np.partition(logits, (E - topk - 1, E - topk), axis=1)
        margin = part[:, E - topk] - part[:, E - topk - 1]
        amb = np.nonzero(margin < DELTA)[0]
        if len(amb):
            logits[amb] = x[amb] @ gate_w.T + gate_b

    lg64 = logits.astype(np.float64)
    pe = np.exp(lg64 - lg64.max(axis=1, keepdims=True))
    probs = pe / pe.sum(axis=1, keepdims=True)
    if topk < E:
        kth = np.partition(logits, E - topk, axis=1)[:, E - topk]
        mask = logits >= kth[:, None]
        wfull = np.where(mask, probs, 0.0).astype(np.float32)
    else:
        wfull = probs.astype(np.float32)

    # ---- host: routing bookkeeping (indices only) ----
    toks = [np.nonzero(wfull[:, e])[0] for e in range(E)]
    counts = np.array([len(t) for t in toks])
    order = np.argsort(-counts, kind="stable")
    # snake assignment: heavy experts paired with light ones so the
    # per-slot capacities C0 >= C1 waste as little padding as possible
    slot_experts = [[] for _ in range(NCORES)]
    for j in range(EPC):
        seg = order[j * NCORES:(j + 1) * NCORES]
        if j % 2 == 1:
            seg = seg[::-1]
        for i in range(NCORES):
            slot_experts[i].append(int(seg[i]))
    Cs = []
    for j in range(EPC):
        cmax = max(counts[slot_experts[i][j]] for i in range(NCORES))
        Cs.append(max(4, int(-(-cmax // 4) * 4)))

    # ---- launch 2: expert matmuls (expert-parallel) ----
    nc2 = _build_expert(Cs, DIN, DOUT, EPC, has_eb)
    wb = expert_w.astype(np_bf16)
    in2 = []
    for i in range(NCORES):
        m = {"wP": np.stack([_pack(wb[e], KT)
                             for e in slot_experts[i]])}
        if has_eb:
            m["bP"] = np.stack(
                [expert_b[e] for e in slot_experts[i]]
            )[:, None, :].astype(np_bf16)
        for j, e in enumerate(slot_experts[i]):
            t = toks[e]
            cnt = len(t)
            xgP = np.zeros((P, KT, Cs[j]), np_bf16)
            if cnt:
                xg = (x[t] * wfull[t, e][:, None]).astype(np_bf16)
                xgP[:, :, :cnt] = _pack(np.ascontiguousarray(xg.T), KT)
            m[f"xgP{j}"] = xgP
            if has_eb:
                gv = np.zeros((1, Cs[j]), np_bf16)
                gv[0, :cnt] = wfull[t, e].astype(np_bf16)
                m[f"gvP{j}"] = gv
        in2.append(m)
    r2 = _run(nc2, in2)

    # ---- host: scatter-add compact outputs (unshard) ----
    y = np.zeros((B, DOUT), np.float32)
    for i in range(NCORES):
        for j, e in enumerate(slot_experts[i]):
            t = toks[e]
            cnt = len(t)
            if cnt:
                yo = r2[i][f"yT{j}"][:, :cnt].astype(np.float32)
                y[t] += yo.T
    return y


# revision 37
# speedup vs baseline: 1.0309x; 1.0309x over previous
"""MoE routing kernel for Trainium2, 8 NeuronCores.

Strategy (two device launches, host does routing bookkeeping):

  Launch 1 (data-parallel gating): each core computes the gating
  logits for its 1/8 token shard as logits^T = gate_w @ x^T with the
  gate weights stationary (16 columns) and tokens moving (N=512),
  in bf16 with f32 PSUM accumulation. Output: raw logits [E, B/8].
  Host: softmax / top-k / capacity planning from the device logits;
  rows whose top-k decision margin is below DELTA are recomputed in
  f32 so routing matches an f32 reference. Tokens are gathered per
  expert and pre-scaled by their gate values (linearity: g*(xW) =
  (g x)W), then packed bf16 in the [P, KT, C] SBUF layout.

  Launch 2 (expert-parallel): experts are paired big-with-small onto
  cores (2 per core) with per-slot capacities C0 >= C1. Per expert,
  the weight tile W[k, n] is stationary (bf16 -> fast weight load)
  and token columns are moving, in n-outer waves of <= 3 PSUM banks
  (k-outer inside a wave so the PE chases the input DMA stream;
  the first expert's first two waves run as one 6-bank group to
  keep the PE fed during the initial stream). Each output row-tile
  y^T[n] [128, C] is written bf16 as soon as its wave finishes, so
  output DMA overlaps compute instead of forming a tail; the host
  scatter-adds the compact outputs into the final [B, DOUT] f32.

All expert FLOPs and the gating matmul run on device; the host does
index bookkeeping, gate-value scaling, and the final unshard.
"""
import numpy as np
from contextlib import ExitStack

from ml_dtypes import bfloat16 as np_bf16

import concourse.mybir as mybir
from concourse import bacc, tile
from concourse.bass_utils import run_bass_kernel_spmd

NCORES = 8
P = 128
NF = 512
F32 = mybir.dt.float32
BF16 = mybir.dt.bfloat16
ACT_COPY = mybir.ActivationFunctionType.Copy

# margin (in logit units) below which the host re-computes a row's
# gating exactly in f32; device logits carry ~2e-3 abs bf16 error
DELTA = 0.02

# test-harness knobs
TRACE = False
LAST_EXEC_NS = []
LAST_RESULTS = {}

_cache = {}


def _warmup_pe(nc, pool, ps_pool, n_mm, tag="ps"):
    """Dummy bf16 matmuls on scratch data, issued at kernel start so the
    PE's HAM clock-gate reaches 2.4 GHz while the input DMAs stream in."""
    wt = pool.tile([P, NF], BF16, name="warm_sb")
    nc.gpsimd.memset(wt[:], 1.0)
    wp = ps_pool.tile([P, NF], F32, name="warm_ps", tag=tag)
    for _ in range(n_mm):
        nc.tensor.matmul(wp[:], wt[:, :P], wt[:], start=True, stop=True)
    return wt, wp


def _warmup_act(nc, pool):
    """Load the scalar engine's activation table (Copy) during the input
    DMA shadow so the first real ACTIVATE doesn't pay the ~1.3us table
    load. Emit AFTER the input dma_starts so it doesn't delay their
    descriptor issue on the scalar engine."""
    aw = pool.tile([1, 8], F32, name="warm_act")
    nc.vector.memset(aw[:], 0.0)
    nc.scalar.activation(aw[:], aw[:], ACT_COPY)


def _build_gating(Bloc, DIN, E):
    """logits^T [E, Bloc] = gate_w @ x^T in bf16, f32 accumulation.

    Inputs : xP  [P, KT, Bloc] bf16 (x^T packed: (p,k,b) = x[b, k*P+p])
             gwP [P, KT, E]    bf16 (gate_w^T packed likewise)
    Output : lgT [E, Bloc] f32
    """
    key = ("gate", Bloc, DIN, E)
    if key in _cache:
        return _cache[key]
    KT = DIN // P
    TT = Bloc // NF
    assert Bloc % NF == 0 and E <= P
    nc = bacc.Bacc("TRN2", target_bir_lowering=False, debug=False,
                   num_devices=NCORES)
    xP = nc.dram_tensor("xP", [P, KT, Bloc], BF16, kind="ExternalInput")
    gwP = nc.dram_tensor("gwP", [P, KT, E], BF16, kind="ExternalInput")
    lgT = nc.dram_tensor("lgT", [E, Bloc], F32, kind="ExternalOutput")

    with tile.TileContext(nc) as tc:
        with ExitStack() as ctx:
            const = ctx.enter_context(tc.tile_pool(name="const", bufs=1))
            ps = ctx.enter_context(tc.tile_pool(name="ps", bufs=1,
                                                space="PSUM"))
            _warmup_pe(nc, const, ps, 8)
            gw_t = const.tile([P, KT, E], BF16)
            nc.scalar.dma_start(gw_t[:], gwP[:])
            # x in two k-halves, one per HWDGE ring, as separate tiles
            # so the matmul dependencies are exact
            H = KT // 2
            x_lo = const.tile([P, H, Bloc], BF16)
            x_hi = const.tile([P, H, Bloc], BF16)
            nc.sync.dma_start(x_lo[:], xP[:, :H])
            nc.scalar.dma_start(x_hi[:], xP[:, H:])
            pss = [ps.tile([E, NF], F32, name=f"g{t}", tag=f"g{t}")
                   for t in range(TT)]
            # k-outer so the PE consumes x chunks in DMA arrival order
            for k in range(KT):
                xt, kk = (x_lo, k) if k < H else (x_hi, k - H)
                for t in range(TT):
                    nc.tensor.matmul(
                        pss[t][:], gw_t[:, k],
                        xt[:, kk, t * NF:(t + 1) * NF],
                        start=(k == 0), stop=(k == KT - 1))
            lg_sb = const.tile([E, Bloc], F32)
            for t in range(TT):
                nc.vector.tensor_copy(lg_sb[:, t * NF:(t + 1) * NF],
                                      pss[t][:])
                nc.sync.dma_start(lgT[:, t * NF:(t + 1) * NF],
                                  lg_sb[:, t * NF:(t + 1) * NF])
    nc.compile()
    _cache[key] = nc
    return nc


def _ksplits(KT):
    """Descriptor schedule for the contraction dim: k0 and k1 alone
    (their DMA-completion receipts fire early, so the PE starts as soon
    as the warmup ends), then pairs."""
    if KT < 4:
        return [(k, 1) for k in range(KT)]
    return [(0, 1), (1, 1)] + [(k, 2) for k in range(2, KT, 2)]


def _xg_ap(entry, kk, c0, cl):
    """Slice a (possibly chunk-split) xg tile entry."""
    if isinstance(entry, tuple):
        xa, xb, ch0 = entry
        if c0 >= ch0:
            return xb[:, kk, c0 - ch0:c0 - ch0 + cl]
        return xa[:, kk, c0:c0 + cl]
    return entry[:, kk, c0:c0 + cl]


def _chunks(C):
    """Split C token columns into near-equal chunks of <= NF (PSUM bank
    limit), multiples of 4 except possibly the last."""
    n = -(-C // NF)
    base = -(-(-(-C // n)) // 4) * 4
    out = []
    c0 = 0
    while c0 < C:
        cl = min(base, C - c0)
        out.append((c0, cl))
        c0 += cl
    return out


def _build_expert(Cs, DIN, DOUT, EPC, has_eb):
    """Per-core expert compute: y_e^T = W_e^T @ xg_e for each of the
    core's EPC experts, where xg_e is the gathered, gate-scaled token
    matrix (capacity Cs[j], transposed). Weights are the stationary
    operand (bf16 -> FWL); tokens move in chunks of <= 512 columns.

    Inputs : xgP{j} [P, KT, Cs[j]] bf16 (packed (p,k,c) = xg[k*P+p, c])
             wP     [EPC, P, KT, DOUT] bf16 (packed likewise)
             bP     [EPC, 1, DOUT] bf16, gvP{j} [1, Cs[j]] bf16 (if bias)
    Output : yT{j}  [DOUT, Cs[j]] bf16
    """
    key = ("exp", tuple(Cs), DIN, DOUT, EPC, has_eb)
    if key in _cache:
        return _cache[key]
    KT = DIN // P
    NT = DOUT // P
    nc = bacc.Bacc("TRN2", target_bir_lowering=False, debug=False,
                   num_devices=NCORES)
    xg_d = [nc.dram_tensor(f"xgP{j}", [P, KT, Cs[j]], BF16,
                           kind="ExternalInput") for j in range(EPC)]
    w_d = nc.dram_tensor("wP", [EPC, P, KT, DOUT], BF16,
                         kind="ExternalInput")
    y_d = [nc.dram_tensor(f"yT{j}", [DOUT, Cs[j]], BF16,
                          kind="ExternalOutput") for j in range(EPC)]
    if has_eb:
        b_d = nc.dram_tensor("bP", [EPC, 1, DOUT], BF16,
                             kind="ExternalInput")
        gv_d = [nc.dram_tensor(f"gvP{j}", [1, Cs[j]], BF16,
                               kind="ExternalInput") for j in range(EPC)]

    with tile.TileContext(nc) as tc:
        with ExitStack() as ctx:
            xgp = ctx.enter_context(tc.tile_pool(name="xg", bufs=2))
            wp = ctx.enter_context(tc.tile_pool(name="w", bufs=2))
            outp = ctx.enter_context(tc.tile_pool(name="out", bufs=2))
            psp = ctx.enter_context(tc.tile_pool(name="ps", bufs=8,
                                                 space="PSUM"))
            warm = ctx.enter_context(tc.tile_pool(name="warm", bufs=1))
            if has_eb:
                bp = ctx.enter_context(tc.tile_pool(name="b", bufs=2))
            _warmup_pe(nc, warm, psp, 8)

            # pre-issue all input DMAs (inputs for expert j+1 stream
            # while expert j computes; outputs are emitted later so the
            # FIFO HWDGE rings never block a later expert's inputs).
            # Per-k descriptors in consumption order so the PE can chase
            # the stream; xg on the sync ring, w split scalar/gpsimd so
            # three queues pull HBM in parallel.
            splits = _ksplits(KT)
            k2s = {}
            for si, (k0, kl) in enumerate(splits):
                for kk in range(kl):
                    k2s[k0 + kk] = (si, kk)
            tiles = []
            for j in range(EPC):
                # per-split tiles: exact deps let the PE chase the DMA
                # stream split by split instead of waiting for the whole
                # tensor; xg on the sync ring, w on the scalar ring,
                # both in consumption order, smallest splits first
                xg_q = []
                w_q = []
                ch0 = _chunks(Cs[j])[0][1]
                for si, (k0, kl) in enumerate(splits):
                    if j == 0 and si == 0:
                        # the very first MM only needs k0 of the first
                        # token chunk: land it (and its completion
                        # receipt) as early as possible
                        xa = xgp.tile([P, kl, ch0], BF16, tag="xg0a",
                                      name=f"xg{j}_0a")
                        nc.sync.dma_start(xa[:],
                                          xg_d[j][:, k0:k0 + kl, :ch0])
                        xb = xgp.tile([P, kl, Cs[j] - ch0], BF16,
                                      tag="xg0b", name=f"xg{j}_0b")
                        nc.sync.dma_start(xb[:],
                                          xg_d[j][:, k0:k0 + kl, ch0:])
                        xg_q.append((xa, xb, ch0))
                    else:
                        xq = xgp.tile([P, kl, Cs[j]], BF16,
                                      tag=f"xg{si}", name=f"xg{j}_{si}")
                        nc.sync.dma_start(xq[:], xg_d[j][:, k0:k0 + kl])
                        xg_q.append(xq)
                    wq = wp.tile([P, kl, DOUT], BF16, tag=f"w{si}",
                                 name=f"w{j}_{si}")
                    nc.scalar.dma_start(wq[:], w_d[j, :, k0:k0 + kl])
                    w_q.append(wq)
                if has_eb:
                    b_t = bp.tile([1, DOUT], BF16, tag="b", name=f"b{j}")
                    nc.gpsimd.dma_start(b_t[:], b_d[j])
                    gv_t = bp.tile([1, Cs[j]], BF16, tag="gv",
                                   name=f"gv{j}")
                    nc.gpsimd.dma_start(gv_t[:], gv_d[j][:])
                    tiles.append((xg_q, w_q, b_t, gv_t))
                else:
                    tiles.append((xg_q, w_q, None, None))
            _warmup_act(nc, warm)

            for j in range(EPC):
                xg_q, w_q, b_t, gv_t = tiles[j]
                C = Cs[j]
                chs = _chunks(C)
                out_t = outp.tile([P, NT, C], BF16, tag="out",
                                  name=f"out{j}")
                ev = 0
                # n-outer waves: each output row-tile finishes early and
                # streams out while later waves compute (no bulk tail).
                # The first expert's first two waves run as one 6-bank
                # group so the PE has 2x the work per arriving k-split
                # while it chases the initial input DMA stream.
                if j == 0 and len(chs) <= 3:
                    groups = [[0, 1]] + [[n] for n in range(2, NT)]
                else:
                    groups = [[n] for n in range(NT)]
                for grp in groups:
                    pss = {n: [psp.tile([P, NF], F32, tag="ps",
                                        name=f"ps{j}_{n}_{ci}")
                               for ci in range(len(chs))]
                           for n in grp}
                    for k in range(KT):
                        si, kk = k2s[k]
                        for n in grp:
                            for ci, (c0, cl) in enumerate(chs):
                                nc.tensor.matmul(
                                    pss[n][ci][:, :cl],
                                    w_q[si][:, kk, n * P:(n + 1) * P],
                                    _xg_ap(xg_q[si], kk, c0, cl),
                                    start=(k == 0),
                                    stop=(k == KT - 1 and not has_eb))
                    # the very last wave's output is on the kernel's
                    # critical tail: fire it per-chunk so each block
                    # leaves as soon as its eviction lands
                    last_wave = (j == EPC - 1 and grp[-1] == NT - 1)
                    for n in grp:
                        for ci, (c0, cl) in enumerate(chs):
                            if has_eb:
                                nc.tensor.matmul(
                                    pss[n][ci][:, :cl],
                                    b_t[:1, n * P:(n + 1) * P],
                                    gv_t[:1, c0:c0 + cl],
                                    start=False, stop=True)
                            dst = out_t[:, n, c0:c0 + cl]
                            if ev % 2 == 0:
                                nc.vector.tensor_copy(dst,
                                                      pss[n][ci][:, :cl])
                            else:
                                nc.scalar.activation(
                                    dst, pss[n][ci][:, :cl], ACT_COPY)
                            ev += 1
                            if last_wave and n == NT - 1:
                                deng = (nc.sync, nc.scalar)[ci % 2]
                                deng.dma_start(
                                    y_d[j][n * P:(n + 1) * P,
                                           c0:c0 + cl], dst)
                        if not (last_wave and n == NT - 1):
                            deng = (nc.sync, nc.scalar, nc.gpsimd)[n % 3]
                            deng.dma_start(y_d[j][n * P:(n + 1) * P, :],
                                           out_t[:, n, :])
    nc.compile()
    _cache[key] = nc
    return nc


def _run(nc, in_maps):
    kw = {}
    if TRACE:
        kw["trace"] = True
    res = run_bass_kernel_spmd(nc, in_maps, list(range(NCORES)), **kw)
    if TRACE:
        LAST_EXEC_NS.append(res.exec_time_ns)
        LAST_RESULTS["last"] = res
    return res.results


def _pack(mat, KT):
    """[D, N] -> [P, KT, N] with (p, k, n) = mat[k*P+p, n]."""
    D, N = mat.shape
    return np.ascontiguousarray(
        mat.reshape(KT, P, N).transpose(1, 0, 2))


def kernel(x, gate_w, gate_b, expert_w, expert_b, topk):
    x = np.ascontiguousarray(np.asarray(x, dtype=np.float32))
    gate_w = np.asarray(gate_w, dtype=np.float32)
    gate_b = np.asarray(gate_b, dtype=np.float32)
    expert_w = np.asarray(expert_w, dtype=np.float32)
    expert_b = np.asarray(expert_b, dtype=np.float32)
    topk = int(topk)

    B, DIN = x.shape
    E, _, DOUT = expert_w.shape
    assert B % (NCORES * NF) == 0 and DIN % P == 0 and DOUT % (4 * P) == 0
    Bloc = B // NCORES
    EPC = E // NCORES
    assert EPC * NCORES == E and E <= P
    KT = DIN // P
    has_eb = bool(np.any(expert_b))

    # ---- launch 1: gating logits (data-parallel over tokens) ----
    nc1 = _build_gating(Bloc, DIN, E)
    gwP = _pack(gate_w.T.astype(np_bf16), KT)
    xb = x.astype(np_bf16)
    in1 = []
    for c in range(NCORES):
        xP = _pack(np.ascontiguousarray(
            xb[c * Bloc:(c + 1) * Bloc].T), KT)
        in1.append({"xP": xP, "gwP": gwP})
    r1 = _run(nc1, in1)
    logits = np.concatenate(
        [r1[c]["lgT"] for c in range(NCORES)], axis=1).T
    logits = np.ascontiguousarray(logits) + gate_b[None, :]

    # ---- host: exact re-gating for ambiguous rows ----
    if topk < E:
        part = np.partition(logits, (E - topk - 1, E - topk), axis=1)
        margin = part[:, E - topk] - part[:, E - topk - 1]
        amb = np.nonzero(margin < DELTA)[0]
        if len(amb):
            logits[amb] = x[amb] @ gate_w.T + gate_b

    lg64 = logits.astype(np.float64)
    pe = np.exp(lg64 - lg64.max(axis=1, keepdims=True))
    probs = pe / pe.sum(axis=1, keepdims=True)
    if topk < E:
        kth = np.partition(logits, E - topk, axis=1)[:, E - topk]
        mask = logits >= kth[:, None]
        wfull = np.where(mask, probs, 0.0).astype(np.float32)
    else:
        wfull = probs.astype(np.float32)

    # ---- host: routing bookkeeping (indices only) ----
    toks = [np.nonzero(wfull[:, e])[0] for e in range(E)]
    counts = np.array([len(t) for t in toks])
    order = np.argsort(-counts, kind="stable")
    # snake assignment: heavy experts paired with light ones so the
    # per-slot capacities C0 >= C1 waste as little padding as possible
    slot_experts = [[] for _ in range(NCORES)]
    for j in range(EPC):
        seg = order[j * NCORES:(j + 1) * NCORES]
        if j % 2 == 1:
            seg = seg[::-1]
        for i in range(NCORES):
            slot_experts[i].append(int(seg[i]))
    Cs = []
    for j in range(EPC):
        cmax = max(counts[slot_experts[i][j]] for i in range(NCORES))
        Cs.append(max(4, int(-(-cmax // 4) * 4)))

    # ---- launch 2: expert matmuls (expert-parallel) ----
    nc2 = _build_expert(Cs, DIN, DOUT, EPC, has_eb)
    wb = expert_w.astype(np_bf16)
    in2 = []
    for i in range(NCORES):
        m = {"wP": np.stack([_pack(wb[e], KT)
                             for e in slot_experts[i]])}
        if has_eb:
            m["bP"] = np.stack(
                [expert_b[e] for e in slot_experts[i]]
            )[:, None, :].astype(np_bf16)
        for j, e in enumerate(slot_experts[i]):
            t = toks[e]
            cnt = len(t)
            xgP = np.zeros((P, KT, Cs[j]), np_bf16)
            if cnt:
                xg = (x[t] * wfull[t, e][:, None]).astype(np_bf16)
                xgP[:, :, :cnt] = _pack(np.ascontiguousarray(xg.T), KT)
            m[f"xgP{j}"] = xgP
            if has_eb:
                gv = np.zeros((1, Cs[j]), np_bf16)
                gv[0, :cnt] = wfull[t, e].astype(np_bf16)
                m[f"gvP{j}"] = gv
        in2.append(m)
    r2 = _run(nc2, in2)

    # ---- host: scatter-add compact outputs (unshard) ----
    y = np.zeros((B, DOUT), np.float32)
    for i in range(NCORES):
        for j, e in enumerate(slot_experts[i]):
            t = toks[e]
            cnt = len(t)
            if cnt:
                yo = r2[i][f"yT{j}"][:, :cnt].astype(np.float32)
                y[t] += yo.T
    return y
